# revision 1
# baseline (speedup 1.0000x reference)
"""Longformer attention TP-sharded Bass kernel for 8 NeuronCores.

Sharding: tensor-parallel over heads. Core d owns heads 2d, 2d+1:
  - Wq/Wk/Wv rows [128d:128(d+1)]  (nn.Linear: q = x @ Wq.T)
  - Wo columns [128d:128(d+1)]
  Each core computes its heads' sparse (windowed+global) attention and a
  full-size out-proj partial; host sums the 8 partials (the "all-reduce").

Device layout (all bf16 compute, fp32 PSUM accumulate):
  xT  [1024h, 4096s]  - x transposed (host prep) so hidden is contraction dim
  qT/kT [128o, 4096s] - head dims on partitions (head A: 0-63, head B: 64-127)
  v   [128s, 32kb, 130] - natural layout per key block, with a ones column per
                          head so the PV matmul also produces the softmax
                          denominator (col 64 / col 129).
  scores are computed transposed [k, q]: softmax sum over k comes out of the
  PE via the ones column; masks are multiplicative 0/1 on exp(scores) (safe:
  scores are O(1) here, no max-subtraction needed).
"""

import os
import numpy as np
import ml_dtypes

S = 4096
HIDDEN = 1024
N_CORES = 8
OC = 128          # out-proj contraction dims (head dims) per core = 2 heads x 64
NQB = S // 128    # 32 query/key blocks
BF16 = ml_dtypes.bfloat16

_CACHE = {}
LAST_RESULTS = None


def _masks_np():
    """Per-group-class multiplicative masks, pre-concatenated along the key
    blocks of one PSUM group, scoresT [k(partition), q(free)] layout.
    Layout [5, 128, 512]:
      0: mid  [row0 | lo | ones | up]   (qb in 2..30)
      1: q1   [lo0  | ones | up | pad]  (qb == 1, width 384)
      2: q31  [row0 | lo | ones | pad]  (qb == 31, width 384)
      3: q0a  [ones | up0 | col0 | col0] (qb == 0, first group)
      4: q0b  [col0 x4]                  (qb == 0, groups 1..7)
    """
    p = np.arange(128)[:, None]   # key index within block
    f = np.arange(128)[None, :]   # query index within block
    ones = np.ones((128, 128), bool)
    m_lo = (f <= p)
    m_lo0 = m_lo | (p == 0)
    m_up = (f >= p)
    m_up0 = m_up | (f == 0)
    m_row0 = np.broadcast_to(p == 0, (128, 128))
    m_col0 = np.broadcast_to(f == 0, (128, 128))
    out = np.zeros((5, 128, 512), bool)
    out[0] = np.concatenate([m_row0, m_lo, ones, m_up], 1)
    out[1, :, :384] = np.concatenate([m_lo0, ones, m_up], 1)
    out[2, :, :384] = np.concatenate([m_row0, m_lo, ones], 1)
    out[3] = np.concatenate([ones, m_up0, m_col0, m_col0], 1)
    out[4] = np.concatenate([m_col0] * 4, 1)
    return out.astype(BF16)


def _mask_idx_for(qb, g0):
    """Mask slot for the group starting at block-list offset g0, or None."""
    if qb == 0:
        return 3 if g0 == 0 else 4
    if qb == 1:
        return 1
    if qb == NQB - 1:
        return 2
    return 0


def _kbs_for(qb):
    """[(key_block, mask_idx or None)] for query block qb."""
    if qb == 0:
        return [(0, None), (1, 3)] + [(kb, 5) for kb in range(2, NQB)]
    if qb == 1:
        return [(0, 1), (1, None), (2, 2)]
    if qb == NQB - 1:
        return [(0, 4), (qb - 1, 0), (qb, None)]
    return [(0, 4), (qb - 1, 0), (qb, None), (qb + 1, 2)]


def _build():
    import concourse.bass as bass
    import concourse.mybir as mybir
    import concourse.tile as tile
    from concourse import bacc

    f32 = mybir.dt.float32
    bf16 = mybir.dt.bfloat16
    Exp = mybir.ActivationFunctionType.Exp

    nc = bacc.Bacc("TRN2", target_bir_lowering=False, debug=False,
                   num_devices=N_CORES)

    xt_d = nc.dram_tensor("xt", [HIDDEN, S], bf16, kind="ExternalInput").ap()
    wq_d = nc.dram_tensor("wqt", [HIDDEN, OC], bf16, kind="ExternalInput").ap()
    wk_d = nc.dram_tensor("wkt", [HIDDEN, OC], bf16, kind="ExternalInput").ap()
    wv_d = nc.dram_tensor("wvt", [HIDDEN, OC], bf16, kind="ExternalInput").ap()
    wo_d = nc.dram_tensor("wot", [OC, HIDDEN], bf16, kind="ExternalInput").ap()
    out_d = nc.dram_tensor("partial", [S, HIDDEN], bf16,
                           kind="ExternalOutput").ap()
    mask_d = nc.inline_tensor(_masks_np(), name="masks").ap()
    id_d = nc.inline_tensor(np.eye(128, dtype=BF16), name="ident").ap()

    with tile.TileContext(nc) as tc:
        import contextlib
        with contextlib.ExitStack() as ctx:
            big = ctx.enter_context(tc.tile_pool(name="big", bufs=1))
            tmp = ctx.enter_context(tc.tile_pool(name="tmp", bufs=3))
            psb = ctx.enter_context(tc.tile_pool(name="psb", bufs=3, space="PSUM"))
            pso = ctx.enter_context(tc.tile_pool(name="pso", bufs=2, space="PSUM"))
            pst = ctx.enter_context(tc.tile_pool(name="pst", bufs=2, space="PSUM"))

            # ---- resident tensors ----
            xt_sb = big.tile([128, 8, S], bf16)       # x.T, hidden chunks on dim1
            qt_sb = big.tile([128, S], bf16)          # q.T * 0.125
            kt_sb = big.tile([128, S], bf16)
            v_sb = big.tile([128, NQB, 130], bf16)    # [vA|1|vB|1] per key block
            outn_sb = big.tile([128, NQB, 128], bf16)  # attn out, natural [q, hd]
            outt_sb = big.tile([128, NQB, 128], bf16)  # transposed [hd, q]
            wq_sb = big.tile([128, 8, OC], bf16)
            wk_sb = big.tile([128, 8, OC], bf16)
            wv_sb = big.tile([128, 8, OC], bf16)
            wo_sb = big.tile([128, HIDDEN], bf16)
            mask_sb = big.tile([128, 5, 512], bf16)
            id_sb = big.tile([128, 128], bf16)

            # ---- constant / weight loads ----
            nc.sync.dma_start(wq_sb, wq_d.rearrange("(c p) o -> p c o", p=128))
            nc.sync.dma_start(wk_sb, wk_d.rearrange("(c p) o -> p c o", p=128))
            nc.sync.dma_start(wv_sb, wv_d.rearrange("(c p) o -> p c o", p=128))
            nc.sync.dma_start(wo_sb, wo_d)
            nc.sync.dma_start(mask_sb, mask_d.rearrange("m p f -> p m f"))
            nc.sync.dma_start(id_sb, id_d)
            nc.vector.memset(v_sb[:, :, 64], 1.0)
            nc.vector.memset(v_sb[:, :, 129], 1.0)

            xt_ap = xt_d.rearrange("(c p) s -> p c s", p=128)

            # ---- phase A: projections ----
            for sc in range(8):
                ssl = slice(sc * 512, (sc + 1) * 512)
                nc.sync.dma_start(xt_sb[:, :, ssl], xt_ap[:, :, ssl])

                psq = psb.tile([128, 512], f32, tag="ps512", name="psq")
                for hc in range(8):
                    nc.tensor.matmul(psq, wq_sb[:, hc, :], xt_sb[:, hc, ssl],
                                     start=(hc == 0), stop=(hc == 7))
                # fold the 1/sqrt(hd) = 0.125 softmax scale into q
                nc.vector.tensor_scalar_mul(qt_sb[:, ssl], psq, 0.125)

                psk = psb.tile([128, 512], f32, tag="ps512", name="psk")
                for hc in range(8):
                    nc.tensor.matmul(psk, wk_sb[:, hc, :], xt_sb[:, hc, ssl],
                                     start=(hc == 0), stop=(hc == 7))
                nc.vector.tensor_copy(kt_sb[:, ssl], psk)

                for b in range(4):
                    kb = sc * 4 + b
                    bsl = slice(sc * 512 + b * 128, sc * 512 + b * 128 + 128)
                    psv = psb.tile([128, 512], f32, tag="ps512", name="psv")
                    for hc in range(8):
                        nc.tensor.matmul(psv[:, :128], xt_sb[:, hc, bsl],
                                         wv_sb[:, hc, :],
                                         start=(hc == 0), stop=(hc == 7))
                    # single strided copy: [vA(64) -> col 0] and [vB -> col 65]
                    vdst = v_sb[:, kb, :].rearrange("p (h c) -> p h c", h=2)
                    nc.vector.tensor_copy(
                        vdst[:, :, 0:64],
                        psv[:, 0:128].rearrange("p (h c) -> p h c", h=2))

            # ---- phase B + C interleaved per query block ----
            for qb in range(NQB):
                qsl = slice(qb * 128, (qb + 1) * 128)
                for h in range(2):
                    bp = 64 * h
                    blocks = _kbs_for(qb)
                    nmm = len(blocks)
                    pso_t = pso.tile([128, 65], f32, tag="psO", name="pso_t")
                    mmi = 0
                    for g0 in range(0, nmm, 4):
                        grp = blocks[g0:g0 + 4]
                        gw = 128 * len(grp)
                        pss = psb.tile([128, 512], f32, tag="ps512", name="pss")
                        for j, (kb, mi) in enumerate(grp):
                            nc.tensor.matmul(
                                pss[:, j * 128:(j + 1) * 128],
                                kt_sb[bp:bp + 64, kb * 128:(kb + 1) * 128],
                                qt_sb[bp:bp + 64, qsl],
                                start=True, stop=True)
                        probs = tmp.tile([128, 512], bf16, tag="probs",
                                         name="probs")
                        nc.scalar.activation(probs[:, :gw], pss[:, :gw], Exp)
                        mig = _mask_idx_for(qb, g0)
                        nc.vector.tensor_mul(probs[:, :gw], probs[:, :gw],
                                             mask_sb[:, mig, :gw])
                        for j, (kb, mi) in enumerate(grp):
                            nc.tensor.matmul(
                                pso_t, probs[:, j * 128:(j + 1) * 128],
                                v_sb[:, kb, 65 * h:65 * h + 65],
                                start=(mmi == 0), stop=(mmi == nmm - 1),
                                skip_group_check=True)
                            mmi += 1
                    recip = tmp.tile([128, 1], f32, tag="recip", name="recip")
                    nc.vector.reciprocal(recip, pso_t[:, 64:65])
                    nc.vector.tensor_scalar_mul(
                        outn_sb[:, qb, 64 * h:64 * h + 64],
                        pso_t[:, 0:64], recip)

                # out-proj for this query block (overlaps later qbs' attention)
                pstr = pst.tile([128, 128], bf16, tag="psT", name="pstr")
                nc.tensor.transpose(pstr, outn_sb[:, qb, :], id_sb)
                nc.vector.tensor_copy(outt_sb[:, qb, :], pstr)
                stage = tmp.tile([128, HIDDEN], bf16, tag="stage", name="stage")
                for oc in range(2):
                    psp = psb.tile([128, 512], f32, tag="ps512", name="psp")
                    nc.tensor.matmul(psp, outt_sb[:, qb, :],
                                     wo_sb[:, oc * 512:(oc + 1) * 512],
                                     start=True, stop=True)
                    if oc == 0:
                        nc.vector.tensor_copy(
                            stage[:, oc * 512:(oc + 1) * 512], psp)
                    else:
                        nc.scalar.copy(stage[:, oc * 512:(oc + 1) * 512], psp)
                nc.sync.dma_start(out_d[qb * 128:(qb + 1) * 128, :], stage)

    nc.compile()
    return nc


def kernel(x, Wq, Wk, Wv, Wo):
    from concourse import bass_utils

    x = np.asarray(x)
    B = x.shape[0]
    xt = np.ascontiguousarray(np.asarray(x)[0].T.astype(BF16))
    in_maps = []
    for d in range(N_CORES):
        rs = slice(OC * d, OC * (d + 1))
        in_maps.append({
            "xt": xt,
            "wqt": np.ascontiguousarray(np.asarray(Wq)[rs, :].T.astype(BF16)),
            "wkt": np.ascontiguousarray(np.asarray(Wk)[rs, :].T.astype(BF16)),
            "wvt": np.ascontiguousarray(np.asarray(Wv)[rs, :].T.astype(BF16)),
            "wot": np.ascontiguousarray(np.asarray(Wo)[:, rs].T.astype(BF16)),
        })

    if "nc" not in _CACHE:
        _CACHE["nc"] = _build()
    nc = _CACHE["nc"]

    res = bass_utils.run_bass_kernel_spmd(
        nc, in_maps, core_ids=list(range(N_CORES)),
        trace=bool(os.environ.get("KERNEL_TRACE")))
    global LAST_RESULTS
    LAST_RESULTS = res

    out = np.zeros((S, HIDDEN), np.float64)
    for r in res.results:
        out += r["partial"].astype(np.float64)
    return out.reshape(B, S, HIDDEN).astype(np.float32)



# revision 2
# speedup vs baseline: 1.5211x; 1.5211x over previous
"""Longformer attention TP-sharded Bass kernel for 8 NeuronCores (v2).

Sharding: tensor-parallel over heads. Core d owns heads 2d, 2d+1:
  - Wq/Wk/Wv rows [128d:128(d+1)]  (nn.Linear: q = x @ Wq.T)
  - Wo columns [128d:128(d+1)]
  Each core computes its heads' sparse (windowed+global) attention and a
  full-size out-proj partial; host sums the 8 partials (the "all-reduce").

v2 layout (all bf16 compute, fp32 PSUM accumulate):
  xT  [1024h, 4096s]   x transposed; DMA'd in 8 contiguous 1MB chunks
  qT/kT [128o, 4096s]  head dims on partitions (head A: 0-63, head B: 64-127)
  v   [128s, 32kb, 130] natural layout per key block: [vA | 1 | vB | 1]; the
                        ones columns make the PV matmul also emit the softmax
                        denominator.
  scoresT [k, q] per (qb, head) in one PSUM bank [128, 512]:
     [prev-block | next-block | diag-block | global-col strip(row 0, M=1 mm)]
  Global query row 0 (qb0) goes through M=1 strip matmuls so only the needed
  row is computed.  Masks are multiplicative 0/1 bf16 on exp(scores), 256
  cols wide, applied on the idle GpSimd engine.  Head A (PE rows 0-63) and
  head B (rows 64-127) score matmuls are emitted adjacently so the PE runs
  them concurrently (row-group tiling).
"""

import os
import numpy as np
import ml_dtypes

S = 4096
HIDDEN = 1024
N_CORES = 8
OC = 128          # out-proj contraction dims (head dims) per core = 2 heads x 64
NQB = S // 128    # 32 query/key blocks
BF16 = ml_dtypes.bfloat16

_CACHE = {}
LAST_RESULTS = None


def _masks_np():
    """Multiplicative masks [4, 128, 256] bf16, scoresT [k(part), q(free)]:
      0: interior qb: [keep f<=p (prev) | keep f>=p (next)]
      1: qb1:         [keep (f<=p)&(p>0) (kb0) | keep f>=p (kb2)]
      2: qb31:        [keep f<=p (kb30) | ones (diag)]
      3: qb0:         [keep (f>=p)|(f==0) (kb1) | ones (kb0)]
    """
    p = np.arange(128)[:, None]   # key index within block
    f = np.arange(128)[None, :]   # query index within block
    ones = np.ones((128, 128), bool)
    m_lo = (f <= p)
    m_lo_ng = m_lo & (p > 0)
    m_up = (f >= p)
    m_up0 = m_up | (f == 0)
    out = np.zeros((4, 128, 256), bool)
    out[0] = np.concatenate([m_lo, m_up], 1)
    out[1] = np.concatenate([m_lo_ng, m_up], 1)
    out[2] = np.concatenate([m_lo, ones], 1)
    out[3] = np.concatenate([m_up0, ones], 1)
    return out.astype(BF16)


def _band_for(qb):
    """[(key_block, col_offset)] band blocks for query block qb (qb >= 1)."""
    if qb == NQB - 1:
        return [(qb - 1, 0), (qb, 128)]
    return [(qb - 1, 0), (qb + 1, 128), (qb, 256)]


def _mask_cls(qb):
    if qb == 1:
        return 1
    if qb == NQB - 1:
        return 2
    return 0


def _build():
    import concourse.bass as bass
    import concourse.mybir as mybir
    import concourse.tile as tile
    from concourse import bacc

    f32 = mybir.dt.float32
    bf16 = mybir.dt.bfloat16
    Exp = mybir.ActivationFunctionType.Exp

    nc = bacc.Bacc("TRN2", target_bir_lowering=False, debug=False,
                   num_devices=N_CORES)

    # xt chunks: [sc][p][c][512] so each chunk is one contiguous 1MB transfer
    xt_d = nc.dram_tensor("xt", [8, 128, 8, 512], bf16, kind="ExternalInput").ap()
    # weights pre-arranged host-side: [p][c][o] contiguous
    wq_d = nc.dram_tensor("wqt", [128, 8, OC], bf16, kind="ExternalInput").ap()
    wk_d = nc.dram_tensor("wkt", [128, 8, OC], bf16, kind="ExternalInput").ap()
    wv_d = nc.dram_tensor("wvt", [128, 8, OC], bf16, kind="ExternalInput").ap()
    wo_d = nc.dram_tensor("wot", [OC, HIDDEN], bf16, kind="ExternalInput").ap()
    out_d = nc.dram_tensor("partial", [S, HIDDEN], bf16,
                           kind="ExternalOutput").ap()
    mask_d = nc.inline_tensor(
        np.ascontiguousarray(_masks_np().transpose(1, 0, 2)), name="masks").ap()
    id_d = nc.inline_tensor(np.eye(128, dtype=BF16), name="ident").ap()

    with tile.TileContext(nc) as tc:
        import contextlib
        with contextlib.ExitStack() as ctx:
            big = ctx.enter_context(tc.tile_pool(name="big", bufs=1))
            tmp = ctx.enter_context(tc.tile_pool(name="tmp", bufs=3))
            psS = ctx.enter_context(tc.tile_pool(name="psS", bufs=3, space="PSUM"))
            psP = ctx.enter_context(tc.tile_pool(name="psP", bufs=2, space="PSUM"))
            psT = ctx.enter_context(tc.tile_pool(name="psT", bufs=1, space="PSUM"))
            psO = ctx.enter_context(tc.tile_pool(name="psO", bufs=2, space="PSUM"))

            # ---- resident tensors ----
            xt_sb = big.tile([128, 8, S], bf16)       # x.T, hidden chunks on dim1
            qt_sb = big.tile([128, S], bf16)          # q.T (0.125 folded in Wq)
            kt_sb = big.tile([128, S], bf16)
            v_sb = big.tile([128, NQB, 130], bf16)    # [vA|1|vB|1] per key block
            outn_sb = big.tile([128, NQB, 128], bf16)  # attn out, natural [q, hd]
            outt_sb = big.tile([128, NQB, 128], bf16)  # transposed [hd, q]
            wq_sb = big.tile([128, 8, OC], bf16)
            wk_sb = big.tile([128, 8, OC], bf16)
            wv_sb = big.tile([128, 8, OC], bf16)
            wo_sb = big.tile([128, HIDDEN], bf16)
            mask_sb = big.tile([128, 4, 256], bf16)
            id_sb = big.tile([128, 128], bf16)

            # ---- loads: wq + xt0 first so Q(sc0) starts asap ----
            nc.sync.dma_start(wq_sb, wq_d)
            nc.sync.dma_start(xt_sb[:, :, 0:512], xt_d[0])
            nc.sync.dma_start(wk_sb, wk_d)
            nc.sync.dma_start(wv_sb, wv_d)
            nc.sync.dma_start(xt_sb[:, :, 512:1024], xt_d[1])
            nc.sync.dma_start(wo_sb, wo_d)
            nc.sync.dma_start(mask_sb, mask_d)
            nc.sync.dma_start(id_sb, id_d)
            nc.gpsimd.memset(v_sb[:, :, 64], 1.0)
            nc.gpsimd.memset(v_sb[:, :, 129], 1.0)

            def attention(qb):
                qsl = slice(qb * 128, (qb + 1) * 128)
                pso = psP.tile([128, 130], f32, tag="pv", name="pso")
                probs = [None, None]
                if qb > 0:
                    band = _band_for(qb)
                    pss = [psS.tile([128, 512], f32, tag="s", name="pss")
                           for _ in range(2)]
                    # head-interleaved band matmuls: A rows 0-63, B rows 64-127
                    # run concurrently in the PE array (distinct row groups)
                    for kb, off in band:
                        for h in range(2):
                            bp = 64 * h
                            nc.tensor.matmul(
                                pss[h][:, off:off + 128],
                                kt_sb[bp:bp + 64, kb * 128:(kb + 1) * 128],
                                qt_sb[bp:bp + 64, qsl],
                                start=True, stop=True)
                    # global key-0 column strip: M=1, row 0 of cols 384:512
                    for h in range(2):
                        bp = 64 * h
                        nc.tensor.matmul(
                            pss[h][0:1, 384:512],
                            kt_sb[bp:bp + 64, 0:1],
                            qt_sb[bp:bp + 64, qsl],
                            start=True, stop=True)
                    cls = _mask_cls(qb)
                    for h in range(2):
                        pr = tmp.tile([128, 512], bf16, tag="probs", name="pr")
                        probs[h] = pr
                        nc.scalar.activation(pr, pss[h], Exp)
                        nc.gpsimd.tensor_mul(pr[:, 0:256], pr[:, 0:256],
                                             mask_sb[:, cls, :])
                    # PV: probs stationary, v moving; ones cols give denoms
                    for h in range(2):
                        hsl = slice(65 * h, 65 * h + 65)
                        n = len(band)
                        for j, (kb, off) in enumerate(band):
                            nc.tensor.matmul(
                                pso[:, hsl], probs[h][:, off:off + 128],
                                v_sb[:, kb, hsl],
                                start=(j == 0), stop=False,
                                skip_group_check=True)
                        # key-0 contribution: K=1 outer product
                        nc.tensor.matmul(
                            pso[:, hsl], probs[h][0:1, 384:512],
                            v_sb[0:1, 0, hsl],
                            start=False, stop=True, skip_group_check=True)
                else:
                    # qb0: band [kb1 | kb0] + far strip (scores of q0 vs kb2..31)
                    pss = [psS.tile([128, 512], f32, tag="s", name="pss")
                           for _ in range(2)]
                    for kb, off in ((1, 0), (0, 128)):
                        for h in range(2):
                            bp = 64 * h
                            nc.tensor.matmul(
                                pss[h][:, off:off + 128],
                                kt_sb[bp:bp + 64, kb * 128:(kb + 1) * 128],
                                qt_sb[bp:bp + 64, qsl],
                                start=True, stop=True)
                    for kb in range(2, NQB):
                        for h in range(2):
                            bp = 64 * h
                            nc.tensor.matmul(
                                pss[h][:, 254 + kb:255 + kb],
                                kt_sb[bp:bp + 64, kb * 128:(kb + 1) * 128],
                                qt_sb[bp:bp + 64, 0:1],
                                start=True, stop=True)
                    for h in range(2):
                        pr = tmp.tile([128, 512], bf16, tag="probs", name="pr")
                        probs[h] = pr
                        nc.scalar.activation(pr[:, 0:286], pss[h][:, 0:286], Exp)
                        nc.gpsimd.tensor_mul(pr[:, 0:128], pr[:, 0:128],
                                             mask_sb[:, 3, 0:128])
                    for h in range(2):
                        hsl = slice(65 * h, 65 * h + 65)
                        for j, (kb, off) in enumerate(((1, 0), (0, 128))):
                            nc.tensor.matmul(
                                pso[:, hsl], probs[h][:, off:off + 128],
                                v_sb[:, kb, hsl],
                                start=(j == 0), stop=False,
                                skip_group_check=True)
                        for kb in range(2, NQB):
                            nc.tensor.matmul(
                                pso[0:1, hsl],
                                probs[h][:, 254 + kb:255 + kb],
                                v_sb[:, kb, hsl],
                                start=False, stop=(kb == NQB - 1),
                                skip_group_check=True)

                # normalize + write outn
                recip = tmp.tile([128, 2], f32, tag="recip", name="recip")
                pso_h = pso.rearrange("p (h c) -> p h c", h=2)
                nc.vector.reciprocal(recip, pso_h[:, :, 64])
                for h in range(2):
                    nc.vector.tensor_scalar_mul(
                        outn_sb[:, qb, 64 * h:64 * h + 64],
                        pso[:, 65 * h:65 * h + 64], recip[:, h:h + 1])

                # transpose -> out-proj -> stage -> DMA
                pstr = psT.tile([128, 128], bf16, tag="tr", name="pstr")
                nc.tensor.transpose(pstr, outn_sb[:, qb, :], id_sb)
                nc.vector.tensor_copy(outt_sb[:, qb, :], pstr)
                stage = tmp.tile([128, HIDDEN], bf16, tag="stage", name="stage")
                for oc in range(2):
                    psp = psO.tile([128, 512], f32, tag="o", name="psp")
                    nc.tensor.matmul(psp, outt_sb[:, qb, :],
                                     wo_sb[:, oc * 512:(oc + 1) * 512],
                                     start=True, stop=True)
                    if oc == 0:
                        nc.vector.tensor_copy(
                            stage[:, oc * 512:(oc + 1) * 512], psp)
                    else:
                        nc.scalar.copy(stage[:, oc * 512:(oc + 1) * 512], psp)
                nc.sync.dma_start(out_d[qb * 128:(qb + 1) * 128, :], stage)

            # ---- projections interleaved with attention ----
            done = 0
            for sc in range(8):
                if 2 <= sc + 1 <= 7:
                    nc.sync.dma_start(
                        xt_sb[:, :, (sc + 1) * 512:(sc + 2) * 512], xt_d[sc + 1])
                ssl = slice(sc * 512, (sc + 1) * 512)

                psq = psS.tile([128, 512], f32, tag="s", name="psq")
                for hc in range(8):
                    nc.tensor.matmul(psq, wq_sb[:, hc, :], xt_sb[:, hc, ssl],
                                     start=(hc == 0), stop=(hc == 7))
                nc.vector.tensor_copy(qt_sb[:, ssl], psq)

                psk = psS.tile([128, 512], f32, tag="s", name="psk")
                for hc in range(8):
                    nc.tensor.matmul(psk, wk_sb[:, hc, :], xt_sb[:, hc, ssl],
                                     start=(hc == 0), stop=(hc == 7))
                nc.scalar.copy(kt_sb[:, ssl], psk)

                psv = psS.tile([128, 512], f32, tag="s", name="psv")
                for b in range(4):
                    bsl = slice(sc * 512 + b * 128, sc * 512 + b * 128 + 128)
                    for hc in range(8):
                        nc.tensor.matmul(psv[:, b * 128:b * 128 + 128],
                                         xt_sb[:, hc, bsl], wv_sb[:, hc, :],
                                         start=(hc == 0), stop=(hc == 7),
                                         skip_group_check=True)
                # one strided copy for all 4 blocks x 2 heads
                vdst = v_sb[:, sc * 4:sc * 4 + 4, :].rearrange(
                    "p b (h c) -> p b h c", h=2)
                vsrc = psv.rearrange("p (b h c) -> p b h c", b=4, h=2)
                nc.vector.tensor_copy(vdst[:, :, :, 0:64], vsrc)

                # attention for query blocks whose K/V coverage is complete
                hi = 4 * sc + 2 if sc < 7 else 31
                while done + 1 <= hi:
                    done += 1
                    attention(done)
            attention(0)

    nc.compile()
    return nc


def kernel(x, Wq, Wk, Wv, Wo):
    from concourse import bass_utils

    x = np.asarray(x)
    B = x.shape[0]
    # xt chunks: [sc, p, c, 512]; hidden h = c*128 + p
    xt = np.ascontiguousarray(
        np.asarray(x)[0].T.astype(BF16).reshape(8, 128, 8, 512)
        .transpose(2, 1, 0, 3))

    def wlayout(W, rs, scale=1.0):
        # W[rs, :].T is [1024 (c p), 128 o] -> [p, c, o]
        wt = (np.asarray(W)[rs, :].T * scale).astype(BF16)
        return np.ascontiguousarray(wt.reshape(8, 128, OC).transpose(1, 0, 2))

    in_maps = []
    for d in range(N_CORES):
        rs = slice(OC * d, OC * (d + 1))
        in_maps.append({
            "xt": xt,
            "wqt": wlayout(Wq, rs, 0.125),
            "wkt": wlayout(Wk, rs),
            "wvt": wlayout(Wv, rs),
            "wot": np.ascontiguousarray(np.asarray(Wo)[:, rs].T.astype(BF16)),
        })

    if "nc" not in _CACHE:
        _CACHE["nc"] = _build()
    nc = _CACHE["nc"]

    res = bass_utils.run_bass_kernel_spmd(
        nc, in_maps, core_ids=list(range(N_CORES)),
        trace=bool(os.environ.get("KERNEL_TRACE")))
    global LAST_RESULTS
    LAST_RESULTS = res

    out = np.zeros((S, HIDDEN), np.float64)
    for r in res.results:
        out += r["partial"].astype(np.float64)
    return out.reshape(B, S, HIDDEN).astype(np.float32)
